# revision 50
# baseline (speedup 1.0000x reference)
"""Causal self-attention (B=2, T=2048, C=1024, H=16, D=64) with RoPE on TRN2.

Sharding: 8 cores = 2 (batch) x 4 (head-groups of 4 heads).
Each core computes qkv projection for its heads, RoPE, causal flash
attention, and a partial o_proj (row-parallel over its heads' dims).
Host gather sums the 4 partial o_proj outputs per batch (the row-parallel
"all-reduce" of the Megatron split) and transposes back to [T, C].

On-chip layout is feature-major (xT = x.T etc.) so every matmul contracts
over the partition dim.  q/k features are de-interleaved (re dims then im
dims per head) by permuting Wqkv columns on the host, which turns RoPE
into 4 block multiplies + 4 block add/subs per 128-row tile.

Attention computes S^T = (K Q^T) tiles directly ([k x q] layout) so the
probabilities come out of exp() already transposed for the P^T @ V
accumulation; softmax denominators are produced by an extra all-ones
column appended to V (row 64 of the PV psum accumulator).  No max
subtraction is needed: logits are O(+-8) here, far below exp overflow.
"""

import sys
import os

sys.path.insert(0, "/opt/trn_rl_repo")

import numpy as np
from contextlib import ExitStack

import concourse.bass as bass
import concourse.bacc as bacc
import concourse.mybir as mybir
import concourse.tile as tile

F32 = mybir.dt.float32
F32R = mybir.dt.float32r

# problem constants (hardcoded per contract)
B, T, C, NH, D = 2, 2048, 1024, 16, 64
HL = 4            # local heads per core
NCORE = 8
CH = 512          # qkv T-chunk width
NCHUNK = T // CH  # 4
QT = 1024         # attention q-tile width
NQT = T // QT     # 2
KB = 128          # attention k-block
SCALE = 1.0 / 8.0  # 1/sqrt(D)

# matmul dtype: float32r streams fp32 at bf16 rate when moving free >= 256
MM_DT = F32R


def _mm(x):
    return x


def _splits(a, b):
    """Split [a, b) at absolute 512 boundaries (psum bank = 512 f32)."""
    out = []
    while a < b:
        nxt = min(b, (a // 512 + 1) * 512)
        out.append((a, nxt))
        a = nxt
    return out


def build_nc():
    nc = bacc.Bacc("TRN2", debug=False, num_devices=NCORE)

    xT_d = nc.dram_tensor("xT", [C, T], F32R, kind="ExternalInput").ap()
    wqk_d = nc.dram_tensor("wqk", [C, 512], F32R, kind="ExternalInput").ap()
    wv_d = nc.dram_tensor("wv", [C, 256], F32R, kind="ExternalInput").ap()
    wo_d = nc.dram_tensor("wo", [256, C], F32R, kind="ExternalInput").ap()
    cosT_d = nc.dram_tensor("cosT", [32, T], F32, kind="ExternalInput").ap()
    sinT_d = nc.dram_tensor("sinT", [32, T], F32, kind="ExternalInput").ap()
    outT_d = nc.dram_tensor("outT", [C, T], F32, kind="ExternalOutput").ap()

    xT_t = xT_d.rearrange("(kt p) t -> kt p t", p=128)    # [8, 128, T]
    wqk_t = wqk_d.rearrange("(kt p) n -> kt p n", p=128)  # [8, 128, 512]
    wv_t = wv_d.rearrange("(kt p) n -> kt p n", p=128)    # [8, 128, 256]
    wo_t = wo_d.rearrange("(kt p) n -> kt p n", p=128)    # [2, 128, C]

    with tile.TileContext(nc) as tc, ExitStack() as ctx:
        const = ctx.enter_context(tc.tile_pool(name="const", bufs=1))
        xcp = ctx.enter_context(tc.tile_pool(name="xcp", bufs=2))
        rtp = ctx.enter_context(tc.tile_pool(name="rtp", bufs=2))
        pp = ctx.enter_context(tc.tile_pool(name="pp", bufs=5))
        nrm = ctx.enter_context(tc.tile_pool(name="nrm", bufs=1))
        psum = ctx.enter_context(tc.tile_pool(name="psum", bufs=2, space="PSUM"))

        # ---- persistent SBUF tensors ----
        # weights in k-block-major single tiles (one big DMA each)
        wqk_all = const.tile([128, 8 * 512], F32R, tag="wqk", name="wqk")
        wqk_v = wqk_all[:].rearrange("p (kt m n) -> p kt m n", m=4, n=128)
        wqk_dv = wqk_d.rearrange("(kt p) (m n) -> p kt m n", p=128, n=128)
        nc.scalar.dma_start(out=wqk_v[:, :, 0], in_=wqk_dv[:, :, 0])
        nc.scalar.dma_start(out=wqk_v[:, :, 1], in_=wqk_dv[:, :, 1])
        wqk_sb = [wqk_all[:, kb * 512:(kb + 1) * 512] for kb in range(8)]

        cc = const.tile([128, T], F32, tag="cc")
        ss = const.tile([128, T], F32, tag="ss")
        nc.scalar.dma_start(out=cc[0:32, :], in_=cosT_d[:])
        nc.scalar.dma_start(out=ss[0:32, :], in_=sinT_d[:])
        nc.scalar.dma_start(out=wqk_v[:, :, 2], in_=wqk_dv[:, :, 2])
        nc.scalar.dma_start(out=wqk_v[:, :, 3], in_=wqk_dv[:, :, 3])
        wv_all = const.tile([128, 8 * 256], F32R, tag="wv", name="wv")
        wv_sb = [wv_all[:, kb * 256:(kb + 1) * 256] for kb in range(8)]

        def load_wv():
            nc.scalar.dma_start(
                out=wv_all[:].rearrange("p (kt n) -> p kt n", n=256),
                in_=wv_d.rearrange("(kt p) n -> p kt n", p=128))
        wo_all = const.tile([128, 2 * C], F32R, tag="wo", name="wo")
        nc.scalar.dma_start(
            out=wo_all[:].rearrange("p (kt n) -> p kt n", n=C),
            in_=wo_d.rearrange("(kt p) n -> p kt n", p=128))
        wo_sb = [wo_all[:, kb * C:(kb + 1) * C] for kb in range(2)]

        # qkT tiles: 0,1 = q (heads 01 / 23), 2,3 = k.  rows per tile:
        # [re_hA(32) im_hA(32) re_hB(32) im_hB(32)] after rope.
        qkT = [const.tile([128, T], F32R, tag=f"qkT{m}", name=f"qkT{m}") for m in range(4)]
        # v tiles, natural layout + ones column per head: [128, 4*65]
        v_sb = [const.tile([128, 4 * 65], F32R, tag=f"v{i}", name=f"v{i}") for i in range(16)]
        ones_f32 = const.tile([128, 1], F32, tag="ones", name="ones")
        nc.gpsimd.memset(ones_f32[:], 1.0)
        for i in range(16):
            ones_ap = v_sb[i][:].rearrange("p (h e) -> p h e", e=65)[:, :, 64]
            nc.vector.tensor_copy(ones_ap, ones_f32[:, 0:1].to_broadcast((128, 4)))
        # y^T tiles: [128, T] x2 (4 heads x 64 dims)
        yT = [const.tile([128, T], F32R, tag=f"yT{kb}", name=f"yT{kb}") for kb in range(2)]

        chunk_xc = {}

        def qkv_chunk(n):
            qkv_chunk_qk(n)
            qkv_chunk_v(n)

        def qkv_chunk_qk(n):
            t0 = n * CH
            xc_all = xcp.tile([128, 8 * CH], F32R, tag="xc", name="xc")
            xc_view = xc_all[:].rearrange("p (kt t) -> p kt t", t=CH)
            xd_view = xT_d[:, t0:t0 + CH].rearrange("(kt p) t -> p kt t", p=128)
            nc.sync.dma_start(out=xc_view[:, 0:4], in_=xd_view[:, 0:4])
            nc.sync.dma_start(out=xc_view[:, 4:8], in_=xd_view[:, 4:8])
            xc = [xc_all[:, kb * CH:(kb + 1) * CH] for kb in range(8)]
            chunk_xc[n] = xc
            # q/k m-tiles: m0 = re dims of all 4 q heads, m1 = im dims,
            # m2/m3 same for k.  rope = 6 full-width DVE ops per q/k,
            # then 16 small copies relayout to head-contiguous qkT.
            mul = mybir.AluOpType.mult
            sub = mybir.AluOpType.subtract
            add = mybir.AluOpType.add
            for r in range(1, 4):
                nc.gpsimd.tensor_copy(
                    cc[32 * r:32 * r + 32, t0:t0 + CH], cc[0:32, t0:t0 + CH])
                nc.gpsimd.tensor_copy(
                    ss[32 * r:32 * r + 32, t0:t0 + CH], ss[0:32, t0:t0 + CH])
            for g in range(2):  # 0 = q, 1 = k
                pre = psum.tile([128, CH], F32, tag="qk", name="psre")
                pim = psum.tile([128, CH], F32, tag="qk", name="psim")
                for ps, m in ((pre, 2 * g), (pim, 2 * g + 1)):
                    for kb in range(8):
                        nc.tensor.matmul(
                            ps[:, 0:CH],
                            lhsT=_mm(wqk_sb[kb][:, m * 128:(m + 1) * 128]),
                            rhs=_mm(xc[kb]),
                            start=(kb == 0),
                            stop=(kb == 7),
                        )
                ccn = cc[:, t0:t0 + CH]
                ssn = ss[:, t0:t0 + CH]
                t1 = rtp.tile([128, CH], F32, tag="t1")
                t2 = rtp.tile([128, CH], F32, tag="t2")
                t3 = rtp.tile([128, CH], F32, tag="t3")
                t4 = rtp.tile([128, CH], F32, tag="t4")
                nc.vector.tensor_tensor(t1[:], pre[:, 0:CH], ccn, mul)
                nc.vector.tensor_tensor(t2[:], pim[:, 0:CH], ssn, mul)
                nc.vector.tensor_tensor(t3[:], pre[:, 0:CH], ssn, mul)
                nc.vector.tensor_tensor(t4[:], pim[:, 0:CH], ccn, mul)
                rall, iall = t1, t3
                nc.vector.tensor_tensor(rall[:], t1[:], t2[:], sub)
                nc.vector.tensor_tensor(iall[:], t3[:], t4[:], add)
                # relayout: head h -> qkT[2*g + h//2] rows 64*(h%2)+[re|im]
                for h in range(4):
                    o = qkT[2 * g + h // 2]
                    r0 = 64 * (h % 2)
                    if h == 2:
                        nc.scalar.copy(
                            o[r0:r0 + 32, t0:t0 + CH], rall[32 * h:32 * h + 32, :])
                        nc.scalar.copy(
                            o[r0 + 32:r0 + 64, t0:t0 + CH], iall[32 * h:32 * h + 32, :])
                        continue
                    eng = nc.vector if h == 0 else nc.gpsimd
                    eng.tensor_copy(
                        o[r0:r0 + 32, t0:t0 + CH], rall[32 * h:32 * h + 32, :])
                    eng.tensor_copy(
                        o[r0 + 32:r0 + 64, t0:t0 + CH], iall[32 * h:32 * h + 32, :])


        def qkv_chunk_v(n):
            t0 = n * CH
            xc = chunk_xc[n]
            # v: natural layout [T-part, d]
            for tb in range(4):
                psv = psum.tile([128, CH], F32, tag="qk", name="psv")
                for kb in range(8):
                    nc.tensor.matmul(
                        psv[:, 0:256],
                        lhsT=_mm(xc[kb][:, tb * 128:(tb + 1) * 128]),
                        rhs=_mm(wv_sb[kb]),
                        start=(kb == 0),
                        stop=(kb == 7),
                    )
                vt = v_sb[4 * n + tb]
                dst = vt[:].rearrange("p (h e) -> p h e", e=65)[:, :, 0:64]
                src = psv[:, 0:256].rearrange("p (h d) -> p h d", d=64)
                nc.scalar.copy(dst, src)

        def attention(h, qt):
            """One head, one q-tile of width QT."""
            q0 = qt * QT
            qtile = qkT[h // 2]
            ktile = qkT[2 + h // 2]
            r0 = 64 * (h % 2)
            psy = psum.tile([65, QT], F32, tag="y", name="psy", bufs=1)
            nkb = 8 * qt + 8
            for kb in range(nkb):
                diag = kb >= 8 * qt
                off = 128 * (kb - 8 * qt) if diag else 0
                pst = psum.tile([128, QT], F32, tag="st", name="pst")
                for (a, b) in _splits(off, QT):
                    nc.tensor.matmul(
                        pst[:, a:b],
                        lhsT=_mm(ktile[r0:r0 + 64, kb * 128:(kb + 1) * 128]),
                        rhs=_mm(qtile[r0:r0 + 64, q0 + a:q0 + b]),
                        start=True,
                        stop=True,
                    )
                P = pp.tile([128, QT], F32R, tag="P")
                nc.scalar.activation(
                    P[:, off:QT], pst[:, off:QT],
                    mybir.ActivationFunctionType.Exp, scale=SCALE)
                if diag:
                    # zero strictly-upper triangle of the leading 128 cols
                    nc.gpsimd.affine_select(
                        out=P[:, off:off + 128],
                        in_=P[:, off:off + 128],
                        compare_op=mybir.AluOpType.is_ge,
                        fill=0.0,
                        base=0,
                        pattern=[[1, 128]],
                        channel_multiplier=-1,
                    )
                for (a, b) in _splits(off, QT):
                    # last writer of psum bank r is diag j = 4r+3
                    j_stop = 4 * (a // 512) + 3
                    nc.tensor.matmul(
                        psy[:, a:b],
                        lhsT=_mm(v_sb[kb][:, h * 65:h * 65 + 65]),
                        rhs=_mm(P[:, a:b]),
                        start=(kb == 0),
                        stop=(diag and (kb - 8 * qt) == j_stop),
                    )
            ybuf = nrm.tile([64, QT], F32, tag="ybuf", bufs=2)
            lrow = nrm.tile([1, QT], F32, tag="lrow", bufs=1)
            rl = nrm.tile([1, QT], F32, tag="rl", bufs=1)
            rlb = nrm.tile([64, QT], F32, tag="rlb", bufs=2)
            # bounce psy to SBUF (frees the psum slot early).  The custom-DVE
            # reciprocal misreads inputs at non-zero partition base on HW, so
            # the denominator row must land in a partition-0 tile.
            nc.vector.tensor_copy(ybuf[:], psy[0:64, :])
            nc.vector.tensor_copy(lrow[:], psy[64:65, :])
            nc.vector.reciprocal_approx_fast(rl[:], lrow[:])
            nc.gpsimd.partition_broadcast(rlb[:], rl[:])
            nc.vector.tensor_tensor(
                yT[h // 2][r0:r0 + 64, q0:q0 + QT],
                ybuf[:], rlb[:], mybir.AluOpType.mult)

        def o_proj(ntp):
            for mo in range(8):
                    ps = psum.tile([128, QT], F32, tag="st")
                    for half in range(2):
                        nt = ntp * 2 + half
                        for kb in range(2):
                            nc.tensor.matmul(
                                ps[:, half * 512:(half + 1) * 512],
                                lhsT=_mm(wo_sb[kb][:, mo * 128:(mo + 1) * 128]),
                                rhs=_mm(yT[kb][:, nt * 512:(nt + 1) * 512]),
                                start=(kb == 0),
                                stop=(kb == 1),
                            )
                    ob = pp.tile([128, QT], F32, tag="P", name="ob")
                    if mo % 2 == 0:
                        nc.vector.tensor_copy(ob[:], ps[:])
                    else:
                        nc.scalar.copy(ob[:], ps[:])
                    ring = nc.scalar if (ntp == 1 and mo % 2 == 1) else nc.sync
                    ring.dma_start(
                        out=outT_d[mo * 128:(mo + 1) * 128,
                                   ntp * QT:(ntp + 1) * QT],
                        in_=ob[:])

        load_wv()
        qkv_chunk(0)
        qkv_chunk(1)
        for h in range(HL):
            attention(h, 0)
        qkv_chunk(2)
        qkv_chunk(3)
        o_proj(0)
        for h in range(HL):
            attention(h, 1)
        o_proj(1)

    nc.compile()
    return nc


def shard_inputs(x, freqs_cos, freqs_sin, Wqkv, Wo):
    """Build the 8 per-core input maps (host-side sharding)."""
    x = np.asarray(x, dtype=np.float32)
    Wqkv = np.asarray(Wqkv, dtype=np.float32)
    Wo = np.asarray(Wo, dtype=np.float32)
    cosT = np.ascontiguousarray(np.asarray(freqs_cos, dtype=np.float32).T)
    sinT = np.ascontiguousarray(np.asarray(freqs_sin, dtype=np.float32).T)
    xTs = [np.ascontiguousarray(x[b].T) for b in range(B)]

    in_maps = []
    for c in range(NCORE):
        b, hg = c // 4, c % 4
        re = [np.arange(g * 64, g * 64 + 64, 2)
              for g in range(4 * hg, 4 * hg + 4)]
        im = [np.arange(g * 64 + 1, g * 64 + 64, 2)
              for g in range(4 * hg, 4 * hg + 4)]
        qcols = np.concatenate(re + im)
        kcols = C + qcols
        wqk = np.ascontiguousarray(Wqkv[:, np.concatenate([qcols, kcols])])
        wv = np.ascontiguousarray(Wqkv[:, 2 * C + hg * 256: 2 * C + hg * 256 + 256])
        wo = np.ascontiguousarray(Wo[hg * 256: hg * 256 + 256, :])
        in_maps.append({
            "xT": xTs[b], "wqk": wqk, "wv": wv, "wo": wo,
            "cosT": cosT, "sinT": sinT,
        })
    return in_maps


_NC_CACHE = None


def _get_nc():
    global _NC_CACHE
    if _NC_CACHE is None:
        _NC_CACHE = build_nc()
    return _NC_CACHE


def run(inputs, trace=False):
    from concourse.bass_utils import run_bass_kernel_spmd

    nc = _get_nc()
    in_maps = shard_inputs(**inputs)
    res = run_bass_kernel_spmd(nc, in_maps, list(range(NCORE)), trace=trace)
    out = np.empty((B, T, C), dtype=np.float32)
    for b in range(B):
        acc = res.results[4 * b]["outT"].astype(np.float32)
        for c in range(4 * b + 1, 4 * b + 4):
            acc = acc + res.results[c]["outT"]
        out[b] = acc.T
    return out, res


def kernel(**inputs):
    out, _ = run(inputs)
    return out
